# revision 1
# baseline (speedup 1.0000x reference)
"""Multi-head causal attention (B=4, T=2048, D=1024, H=16) on 8 NeuronCores.

Sharding: data-parallel over batch (4) x tensor-parallel over heads (2 groups
of 8 heads) = 8 cores. Each core runs the QKV projection for its head group
and causal flash-attention. The head-mixing reduction for the output
projection happens on device: an AllToAll inside each batch pair swaps
attention outputs so each core holds every head's features for half the
sequence, then computes that half's full output projection. The host only
concatenates disjoint row slices.

All matmuls run in float32r (TF32-like: full rate on TRN2 for free dim >= 256,
~1e-4 relative error). Scores are computed transposed, S^T[s, tq], so the
softmax normalizer comes free from a ones-column appended to V, and the
1/sqrt(dh) score scale folds into the ACT exp's scale argument. Only
lower-triangle score blocks are computed; s-blocks are processed in pairs
sharing one [128, 1024] PSUM tile so each non-diagonal pair needs a single
exp instruction (ACT's ~352-cycle per-op overhead would otherwise rival the
exp work itself). Diagonal blocks get an additive triangular mask and a
narrowed matmul/exp width.
"""
import sys

sys.path.insert(0, '/opt/trn_rl_repo')

import numpy as np

import concourse.mybir as mybir
import concourse.tile as tile
from concourse import bacc
from concourse.bass_utils import run_bass_kernel_spmd

B, T, D = 4, 2048, 1024
H, DH = 16, 64
HG = 8              # heads per core
GD = HG * DH        # 512 features per core
P = 128
CH = 512            # tq chunk width (one psum bank)
NB = T // P         # 16 s-blocks
NCH = T // CH       # 4 tq chunks
KB = D // P         # 8 contraction blocks over d_model
TH = T // 2         # tq half owned after the AllToAll
F32 = mybir.dt.float32
F32R = mybir.dt.float32r

_CACHE = {}


def build_nc(repeat=1, phases=3):
    nc = bacc.Bacc("TRN2", target_bir_lowering=False, debug=False)

    xT = nc.dram_tensor("xT", [D, T], F32R, kind="ExternalInput")
    wq = nc.dram_tensor("wq", [D, GD], F32R, kind="ExternalInput")
    wk = nc.dram_tensor("wk", [D, GD], F32R, kind="ExternalInput")
    wv = nc.dram_tensor("wv", [D, GD], F32R, kind="ExternalInput")
    wout = nc.dram_tensor("wout", [GD, D], F32R, kind="ExternalInput")
    tril = nc.dram_tensor("tril", [P, P], mybir.dt.bfloat16, kind="ExternalInput")
    ident = nc.dram_tensor("ident", [P, P], mybir.dt.bfloat16, kind="ExternalInput")
    onecol = nc.dram_tensor("onecol", [P, 1], F32R, kind="ExternalInput")
    y = nc.dram_tensor("y", [T, CH], F32, kind="ExternalOutput")

    # Own attention output O^T (features x tq), bounced through DRAM so the
    # output projection can read it as 128-partition feature blocks.
    ot_dram = nc.dram_tensor("ot_own", [GD, T], F32R)
    # ReduceScatter buffers: partial y summed within the batch pair; each
    # core keeps its tq half.
    rs_in = nc.dram_tensor("y_rs_in", [2, T, CH], F32)
    rs_out = nc.dram_tensor("y_rs_out", [T, CH], F32)

    with tile.TileContext(nc) as tc:
        with tc.tile_pool(name="big", bufs=1) as big, \
             tc.tile_pool(name="wres", bufs=1) as wres, \
             tc.tile_pool(name="cst", bufs=1) as cst, \
             tc.tile_pool(name="wstr", bufs=1) as wstr, \
             tc.tile_pool(name="wo", bufs=1) as wop, \
             tc.tile_pool(name="qk", bufs=2) as qkp, \
             tc.tile_pool(name="vp", bufs=1) as vput, \
             tc.tile_pool(name="exps", bufs=4) as expp, \
             tc.tile_pool(name="sml", bufs=2) as sml, \
             tc.tile_pool(name="ystg", bufs=3) as ystg, \
             tc.tile_pool(name="ps_a", bufs=2, space="PSUM") as ps_a, \
             tc.tile_pool(name="ps_s", bufs=2, space="PSUM") as ps_s, \
             tc.tile_pool(name="ps_o", bufs=2, space="PSUM") as ps_o:

            # ---- resident loads ----
            wv_sb = wres.tile([P, KB, GD], F32R, tag="wv")
            wv_r = wv.ap().rearrange("(ko p) n -> p ko n", p=P)
            for k in range(0, KB, 2):
                nc.sync.dma_start(wv_sb[:, k:k + 2], wv_r[:, k:k + 2])
            wo_r = wout.ap().rearrange("(ko p) n -> p ko n", p=P)
            tril_sb = cst.tile([P, P], mybir.dt.bfloat16)
            nc.sync.dma_start(tril_sb[:], tril.ap())
            id_sb = cst.tile([P, P], mybir.dt.bfloat16)
            nc.sync.dma_start(id_sb[:], ident.ap())
            one_sb = cst.tile([P, 1], F32R)
            nc.sync.dma_start(one_sb[:], onecol.ap())

            for _rep in range(repeat):
                xT_sb = big.tile([P, KB, T], F32R, tag="big")
                xT_r = xT.ap().rearrange("(ko p) t -> p ko t", p=P)
                for k in range(KB):
                    nc.sync.dma_start(xT_sb[:, k], xT_r[:, k])

                # ---- V projection; V_aug[:, nb, h, 0:64] = v, [.., 64] = 1
                v_aug = vput.tile([P, NB, HG, DH + 1], F32R)
                nc.vector.tensor_copy(
                    out=v_aug[:, :, :, DH:DH + 1],
                    in_=one_sb[:, :, None, None].to_broadcast((P, NB, HG, 1)),
                )
                for sb_i in range(NB):
                    psum = ps_a.tile([P, CH], F32, tag="proj")
                    for k in range(KB):
                        nc.tensor.matmul(
                            psum[:],
                            lhsT=xT_sb[:, k, sb_i * P:(sb_i + 1) * P],
                            rhs=wv_sb[:, k, :],
                            start=(k == 0), stop=(k == KB - 1),
                        )
                    nc.vector.tensor_copy(
                        out=v_aug[:, sb_i, :, 0:DH],
                        in_=psum.rearrange("p (h d) -> p h d", h=HG),
                    )

                # ---- per head-pair: project Q^T/K^T, then attend ----
                for hh in range(HG // 2):
                    wq_sb = wstr.tile([P, KB, P], F32R, tag="wq")
                    wk_sb = wstr.tile([P, KB, P], F32R, tag="wk")
                    nc.sync.dma_start(
                        wq_sb[:],
                        wq.ap()[:, hh * P:(hh + 1) * P].rearrange(
                            "(ko p) n -> p ko n", p=P))
                    nc.sync.dma_start(
                        wk_sb[:],
                        wk.ap()[:, hh * P:(hh + 1) * P].rearrange(
                            "(ko p) n -> p ko n", p=P))
                    qt = qkp.tile([P, T], F32R, tag="qt")
                    kt = qkp.tile([P, T], F32R, tag="kt")
                    for c4 in range(T // CH):
                        for (w_sb, dst) in ((wq_sb, qt), (wk_sb, kt)):
                            psum = ps_a.tile([P, CH], F32, tag="proj")
                            for k in range(KB):
                                nc.tensor.matmul(
                                    psum[:],
                                    lhsT=w_sb[:, k, :],
                                    rhs=xT_sb[:, k, c4 * CH:(c4 + 1) * CH],
                                    start=(k == 0), stop=(k == KB - 1),
                                )
                            nc.vector.tensor_copy(
                                out=dst[:, c4 * CH:(c4 + 1) * CH], in_=psum[:])

                    for h2 in range(2 if phases >= 2 else 0):
                        h = hh * 2 + h2
                        pb = h2 * DH   # partition base of head in qt/kt
                        ot_stage = sml.tile([DH, T], F32R, tag="otsb")
                        for c in range(NCH):
                            ot_ps = ps_o.tile([DH + 1, CH], F32, tag="ot")
                            nblk = (c + 1) * (CH // P)
                            for ip in range(0, nblk, 2):
                                s_ps = ps_s.tile([P, 2 * CH], F32, tag="s")
                                e_sb = expp.tile([P, 2 * CH], F32R, tag="e")
                                spans = []
                                for o in range(2):
                                    i = ip + o
                                    r = i - c * (CH // P)
                                    f0 = P * r if r >= 0 else 0
                                    base = o * CH
                                    spans.append((i, f0, base))
                                    nc.tensor.matmul(
                                        s_ps[:, base + f0:base + CH],
                                        lhsT=kt[pb:pb + DH, i * P:(i + 1) * P],
                                        rhs=qt[pb:pb + DH,
                                               c * CH + f0:(c + 1) * CH],
                                        start=True, stop=True,
                                    )
                                    if r >= 0:
                                        nc.tensor.matmul(
                                            s_ps[:, base + f0:base + f0 + P],
                                            lhsT=id_sb[:],
                                            rhs=tril_sb[:],
                                            start=False, stop=True,
                                            skip_group_check=True,
                                        )
                                if ip >= c * (CH // P):   # diagonal pair
                                    for (i, f0, base) in spans:
                                        nc.scalar.activation(
                                            e_sb[:, base + f0:base + CH],
                                            s_ps[:, base + f0:base + CH],
                                            mybir.ActivationFunctionType.Exp,
                                            scale=float(DH ** -0.5))
                                else:
                                    nc.scalar.activation(
                                        e_sb[:], s_ps[:],
                                        mybir.ActivationFunctionType.Exp,
                                        scale=float(DH ** -0.5))
                                for (i, f0, base) in spans:
                                    nc.tensor.matmul(
                                        ot_ps[:, f0:CH],
                                        lhsT=v_aug[:, i, h, :],
                                        rhs=e_sb[:, base + f0:base + CH],
                                        start=(i == 0), stop=(i == nblk - 1),
                                    )
                            recip = sml.tile([1, CH], F32, tag="recip")
                            nc.vector.reciprocal(recip[:], ot_ps[DH:DH + 1, :])
                            bcast = sml.tile([DH, CH], F32, tag="bcast")
                            nc.gpsimd.partition_broadcast(bcast[:], recip[:])
                            nc.vector.tensor_mul(
                                out=ot_stage[:, c * CH:(c + 1) * CH],
                                in0=ot_ps[0:DH, :], in1=bcast[:])
                        nc.sync.dma_start(
                            ot_dram.ap()[h * DH:(h + 1) * DH, :], ot_stage[:])

                if phases < 3:
                    z_sb = ystg.tile([P, CH], F32)
                    nc.vector.memset(z_sb[:], 0.0)
                    for m in range(TH // P):
                        for half in range(2):
                            nc.sync.dma_start(
                                y.ap()[m * P:(m + 1) * P,
                                       half * CH:(half + 1) * CH], z_sb[:])
                    continue

                # ---- partial output projection over own 512 features ----
                KO = GD // P   # 4 feature blocks
                ot_own = big.tile([P, KO, T], F32R, tag="big")
                ot_r = ot_dram.ap().rearrange("(ko p) t -> p ko t", p=P)
                for k in range(KO):
                    nc.sync.dma_start(ot_own[:, k], ot_r[:, k])
                for half in range(2):
                    wo_h = wop.tile([P, KO, CH], F32R, tag="woh")
                    nc.sync.dma_start(
                        wo_h[:], wo_r[:, :, half * CH:(half + 1) * CH])
                    for m in range(T // P):
                        psum = ps_a.tile([P, CH], F32, tag="proj")
                        for k in range(KO):
                            nc.tensor.matmul(
                                psum[:],
                                lhsT=ot_own[:, k, m * P:(m + 1) * P],
                                rhs=wo_h[:, k, :],
                                start=(k == 0), stop=(k == KO - 1),
                            )
                        y_sb = ystg.tile([P, CH], F32)
                        nc.vector.tensor_copy(out=y_sb[:], in_=psum[:])
                        nc.sync.dma_start(
                            rs_in.ap()[half, m * P:(m + 1) * P, :], y_sb[:])
                # ReduceScatter over column halves: rank j of each pair
                # keeps sum(partials)[:, j*512:(j+1)*512]
                nc.gpsimd.collective_compute(
                    "ReduceScatter",
                    mybir.AluOpType.add,
                    replica_groups=[[0, 1], [2, 3], [4, 5], [6, 7]],
                    ins=[rs_in.ap()],
                    outs=[rs_out.ap()],
                )
                for q in range(4):
                    nc.sync.dma_start(
                        y.ap()[q * T // 4:(q + 1) * T // 4, :],
                        rs_out.ap()[q * T // 4:(q + 1) * T // 4, :])

    nc.compile()
    return nc


def _get_nc():
    if 'nc' not in _CACHE:
        _CACHE['nc'] = build_nc()
    return _CACHE['nc']


def _make_in_maps(inputs):
    x = np.asarray(inputs["x"], dtype=np.float32)
    W_qkv = np.asarray(inputs["W_qkv"], dtype=np.float32)
    W_out = np.asarray(inputs["W_out"], dtype=np.float32)

    import ml_dtypes
    tril_m = np.where(
        np.arange(P)[:, None] <= np.arange(P)[None, :], 0.0, -1e30
    ).astype(ml_dtypes.bfloat16)
    ident = np.eye(P, dtype=ml_dtypes.bfloat16)
    ones = np.ones((P, 1), np.float32)

    in_maps = []
    for core in range(8):
        b, g = core // 2, core % 2
        in_maps.append({
            "xT": np.ascontiguousarray(x[b].T),
            "wq": np.ascontiguousarray(W_qkv[:, g * GD:(g + 1) * GD]),
            "wk": np.ascontiguousarray(W_qkv[:, D + g * GD:D + (g + 1) * GD]),
            "wv": np.ascontiguousarray(
                W_qkv[:, 2 * D + g * GD:2 * D + (g + 1) * GD]),
            "wout": np.ascontiguousarray(W_out[g * GD:(g + 1) * GD, :]),
            "tril": tril_m,
            "ident": ident,
            "onecol": ones,
        })
    return in_maps


def kernel(x, W_qkv, W_out, mask):
    """Full inputs in, full output out. mask is the known causal tril."""
    in_maps = _make_in_maps({"x": x, "W_qkv": W_qkv, "W_out": W_out})
    nc = _get_nc()
    res = run_bass_kernel_spmd(nc, in_maps, core_ids=list(range(8)))

    out = np.empty((B, T, D), dtype=np.float32)
    for core in range(8):
        b, g = core // 2, core % 2
        out[b, :, g * CH:(g + 1) * CH] = res.results[core]["y"]
    return out

